# revision 16
# baseline (speedup 1.0000x reference)
"""Causal adaptive-kernel attention on 8 TRN2 NeuronCores (Bass/Tile).

Sharding: core i handles batch b = i//2 and heads 8*(i%2) .. 8*(i%2)+8
(d columns 512*(i%2) .. +512).  The per-(b,h) attention maps are computed
in a TRANSPOSED layout S^T[kj, qi] so that:
  - qk^T, the 3x3 conv (as 3 banded matmuls over the kj axis, with the
    qi shifts free via access-pattern offsets), the -1e9 causal mask add,
    and P@V all run on the TensorEngine,
  - the softmax denominators come for free from a ones-column appended
    to V (row 64 of the PV accumulator),
  - the output lands pre-transposed as o^T = the exact lhsT layout the
    final projection needs.
The per-(b,h) 3x3 kernels are generated on-device (hist->context->kernel
MLP, layernorm/gelu/softmax on ACT+DVE) and turned into banded matrices
via scalar_tensor_tensor against constant shifted-identity masks.
The two cores sharing a batch pair-ReduceScatter their projection
partials; the host concatenates the row halves.
"""
import numpy as np
import ml_dtypes
from contextlib import ExitStack

import os
import concourse.bass as bass
import concourse.bacc as bacc
import concourse.tile as tile
from concourse import mybir
from concourse.bass_utils import run_bass_kernel_spmd

F32 = mybir.dt.float32
BF16 = mybir.dt.bfloat16
AF = mybir.ActivationFunctionType
OP = mybir.AluOpType

B, T, D, H, hd, Th, C = 4, 1024, 1024, 16, 64, 256, 2048
CH, NCH, QW = 126, 9, 256            # conv chunk stride, #chunks, quarter width
N_CORES = 8
KTP = 1136                            # padded kT row length (col = tok+1)

bf16 = ml_dtypes.bfloat16


def _chunk_rows(c):
    return min(CH, T - CH * c)


def _tables():
    mask_pairs, mbig_pairs, qstart = [], [], {}
    for c in range(NCH):
        M_c = _chunk_rows(c)
        qs = None
        for Q in range(4):
            qi = np.arange(QW * Q, QW * Q + QW)
            kj_out = CH * c + np.arange(M_c)
            msk = qi[None, :] < kj_out[:, None]
            if msk.all():
                continue
            if qs is None:
                qs = Q
            kj_z = CH * c - 1 + np.arange(128)
            if (qi[None, :] < kj_z[:, None]).any():
                mask_pairs.append((c, Q))
            if msk.any():
                mbig_pairs.append((c, Q))
        qstart[c] = qs
    return mask_pairs, mbig_pairs, qstart


MASK_PAIRS, MBIG_PAIRS, QSTART = _tables()
NMQ = len(MASK_PAIRS)


def _host_consts():
    i3 = np.zeros((128, 3 * 128), np.float32)
    for d in range(3):
        idx = np.arange(128 - d)
        i3[idx + d, 128 * d + idx] = 1.0
    i1f = np.eye(128, dtype=np.float32)
    maskq = np.zeros((128, NMQ * QW), np.float32)
    for n, (c, Q) in enumerate(MASK_PAIRS):
        kj_z = CH * c - 1 + np.arange(128)
        qi = np.arange(QW * Q, QW * Q + QW)
        maskq[:, n * QW:(n + 1) * QW] = (qi[None, :] >= kj_z[:, None])
    mbig = np.zeros((128, NMQ * QW), np.float32)
    for n, (c, Q) in enumerate(MBIG_PAIRS):
        kj_out = CH * c + np.arange(128)      # rows >= M_c unused
        qi = np.arange(QW * Q, QW * Q + QW)
        mbig[:, n * QW:(n + 1) * QW] = np.where(
            qi[None, :] < kj_out[:, None], -1e9, 0.0)
    return i3.astype(bf16), i1f, maskq.astype(bf16), mbig.astype(bf16)


_CACHE = {}


def _build():
    if "nc" in _CACHE:
        return _CACHE["nc"]
    nc = bacc.Bacc("TRN2", target_bir_lowering=False, debug=False,
                   num_devices=N_CORES)

    def din(name, shape, dt=BF16):
        return nc.dram_tensor(name, shape, dt, kind="ExternalInput").ap()

    xt_d = din("xt", [D, T])                      # x[b].T
    wq_d = din("wq", [D, 512])                    # Wq[:, cols] / 8
    wk_d = din("wk", [D, 512])
    wv_d = din("wv", [D, 512])
    pw_d = din("pw", [512, D])                    # proj_W[cols, :]
    ht_d = din("ht", [D, Th])                     # hist[b].T
    hw_d = din("hw", [D, C])                      # hist_W
    c1_d = din("c1", [C, 512])                    # ctx_W1
    c2_d = din("c2", [512, 1])                    # ctx_W2
    k1_d = din("k1", [C, D])                      # kg_W1
    k2_d = din("k2", [D, 72])                     # kg_W2[:, my 72]
    i3_d = din("i3", [128, 3 * 128])
    i1f_d = din("i1f", [128, 128], F32)
    mq_d = din("mq", [128, NMQ * QW])
    mb_d = din("mb", [128, NMQ * QW])

    out_d = nc.dram_tensor("out", [512, D], F32, kind="ExternalOutput").ap()
    DBG = bool(os.environ.get("BASSDBG"))
    dbg = {}
    if DBG:
        for nm, shape, dt in [("dqt", [128, 4 * T], BF16), ("dktp", [128, 4 * KTP], BF16),
                              ("deh", [128, 2 * C], BF16), ("dkpn", [1, 72], F32),
                              ("doT", [128, 4 * T], BF16), ("dvpp", [128, NCH * 520], BF16),
                              ("dz0", [128, T], BF16), ("de0", [128, T], BF16),
                              ("dov", [65, T], F32), ("dpart", [T, D], BF16)]:
            dbg[nm] = nc.dram_tensor(nm, shape, dt, kind="ExternalOutput").ap()
    part_d = nc.dram_tensor("part", [T, D], BF16)
    crs_d = nc.dram_tensor("crs", [512, D], BF16)

    with tile.TileContext(nc) as tc, ExitStack() as ctx:
        pc = ctx.enter_context(tc.tile_pool(name="pc", bufs=1))
        pwp = ctx.enter_context(tc.tile_pool(name="pwp", bufs=1))
        ph = ctx.enter_context(tc.tile_pool(name="ph", bufs=1))
        pks = ctx.enter_context(tc.tile_pool(name="pks", bufs=4))
        pz = ctx.enter_context(tc.tile_pool(name="pz", bufs=2))
        pe_ = ctx.enter_context(tc.tile_pool(name="pe", bufs=2))
        pbd = ctx.enter_context(tc.tile_pool(name="pbd", bufs=2))
        pn = ctx.enter_context(tc.tile_pool(name="pn", bufs=2))
        ppt = ctx.enter_context(tc.tile_pool(name="ppt", bufs=2))
        ppq = ctx.enter_context(tc.tile_pool(name="ppq", bufs=1, space="PSUM"))
        ppk = ctx.enter_context(tc.tile_pool(name="ppk", bufs=2, space="PSUM"))
        ppc = ctx.enter_context(tc.tile_pool(name="ppc", bufs=2, space="PSUM"))
        ppv = ctx.enter_context(tc.tile_pool(name="ppv", bufs=1, space="PSUM"))

        # ---------------- constant / weight loads ----------------
        i3 = pc.tile([128, 3 * 128], BF16); nc.sync.dma_start(i3[:], i3_d[:])
        i1f = pc.tile([128, 128], F32); nc.sync.dma_start(i1f[:], i1f_d[:])
        mq = pc.tile([128, NMQ * QW], BF16); nc.sync.dma_start(mq[:], mq_d[:])
        mb = pc.tile([128, NMQ * QW], BF16); nc.sync.dma_start(mb[:], mb_d[:])
        ones_f = pc.tile([128, 1], F32); nc.vector.memset(ones_f[:], 1.0)
        ones_r = pc.tile([1, 128], F32); nc.vector.memset(ones_r[:], 1.0)
        eps_t = pc.tile([128, 1], F32); nc.vector.memset(eps_t[:], 1e-5)

        def load_kp(dram, kchunks, m, nm):
            t = pwp.tile([128, kchunks * m], BF16, name=nm)
            nc.sync.dma_start(
                t[:].rearrange("p (k m) -> p k m", k=kchunks),
                dram.rearrange("(k p) m -> p k m", p=128))
            return t

        xt = load_kp(xt_d, 8, T, "xt_t")          # [128, 8*1024]
        wq = load_kp(wq_d, 8, 512, "wq_t")
        wk = load_kp(wk_d, 8, 512, "wk_t")
        wv = load_kp(wv_d, 8, 512, "wv_t")
        pw = load_kp(pw_d, 4, D, "pw_t")          # [128, 4*1024]
        ht = load_kp(ht_d, 8, Th, "ht_t")         # [128, 8*256]
        k2 = load_kp(k2_d, 8, 72, "k2_t")
        c2 = pwp.tile([128, 4], BF16)
        nc.sync.dma_start(c2[:].rearrange("p (k m) -> p k m", k=4),
                          c2_d.rearrange("(k p) m -> p k m", p=128))

        # ---------------- kg path: eh = gelu(ln(hist @ hist_W)) ----------------
        eh = ph.tile([128, 2 * C], BF16)           # tok chunk m at cols 2048m
        ehT = ph.tile([128, 16 * Th], BF16)        # C chunk k at cols 256k
        zeh = ph.tile([128, C], F32)
        stats = ph.tile([128, 8], F32)             # accum parts / mean / var...
        rstd2 = ph.tile([128, 2], F32)
        xc = ph.tile([128, C], F32)
        for m in range(2):
            acc = stats
            zp = ppc.tile([128, 1024], F32, tag='c')
            for half in range(2):                  # 2 psum rounds of 2x512
                for n2 in range(2):
                    n = 2 * half + n2
                    for k in range(8):
                        rhs = pks.tile([128, 512], BF16, tag="hwc")
                        nc.sync.dma_start(rhs[:], hw_d[128 * k:128 * k + 128,
                                                       512 * n:512 * n + 512])
                        nc.tensor.matmul(zp[:, 512 * n2:512 * n2 + 512],
                                         ht[:, 256 * k + 128 * m:256 * k + 128 * m + 128],
                                         rhs[:], start=(k == 0), stop=(k == 7),
                                         skip_group_check=True)
                    nc.scalar.activation(
                        zeh[:, 512 * n:512 * n + 512],
                        zp[:, 512 * n2:512 * n2 + 512], AF.Copy,
                        accum_out=acc[:, n:n + 1])
                if half == 0:
                    zp = ppc.tile([128, 1024], F32, tag='c')
            nc.vector.tensor_reduce(stats[:, 4:5], acc[:, 0:4], mybir.AxisListType.X, OP.add)
            nc.vector.tensor_scalar(stats[:, 5:6], stats[:, 4:5], -1.0 / C, None, OP.mult)
            nc.scalar.activation(xc[:], zeh[:], AF.Identity, bias=stats[:, 5:6])
            nc.scalar.activation(zeh[:], xc[:], AF.Square, accum_out=stats[:, 6:7])
            nc.vector.tensor_scalar(stats[:, 7:8], stats[:, 6:7], 1.0 / C, None, OP.mult)
            nc.scalar.activation(stats[:, 0:1], stats[:, 7:8], AF.Ln, bias=eps_t[:, 0:1])
            nc.scalar.activation(rstd2[:, m:m + 1], stats[:, 0:1], AF.Exp, scale=-0.5)
            nc.scalar.activation(eh[:, C * m:C * m + C], xc[:], AF.Gelu,
                                 scale=rstd2[:, m:m + 1])
        # ehT via block DMA transposes
        for m in range(2):
            for k in range(16):
                nc.sync.dma_start(ehT[:, 256 * k + 128 * m:256 * k + 128 * m + 128],
                                  eh[:, C * m + 128 * k:C * m + 128 * k + 128],
                                  transpose=True)
        # g1T = ctx_W1^T @ ehT   [512(4 chunks), 256]
        g1gT = ph.tile([128, 4 * Th], BF16)
        g1p = ppq.tile([128, 1024], F32, tag='q')
        for k in range(16):
            c1c = pks.tile([128, 512], BF16, tag="hwc")
            nc.sync.dma_start(c1c[:], c1_d[128 * k:128 * k + 128, :])
            for mj in range(4):
                nc.tensor.matmul(g1p[:, 256 * mj:256 * mj + 256],
                                 c1c[:, 128 * mj:128 * mj + 128],
                                 ehT[:, 256 * k:256 * k + 256],
                                 start=(k == 0 and mj % 2 == 0),
                                 stop=(k == 15 and mj % 2 == 1),
                                 skip_group_check=True)
        nc.scalar.activation(g1gT[:], g1p[:], AF.Gelu)
        # aw logits [256, 1] -> softmax over Th
        awp = ppc.tile([128, 1024], F32, tag='c')
        for m2 in range(2):
            for u in range(4):
                nc.tensor.matmul(awp[:, m2:m2 + 1],
                                 g1gT[:, 256 * u + 128 * m2:256 * u + 128 * m2 + 128],
                                 c2[:, u:u + 1],
                                 start=(u == 0 and m2 == 0), stop=(u == 3 and m2 == 1),
                                 skip_group_check=True)
        aw_e = ph.tile([128, 2], F32)
        nc.scalar.activation(aw_e[:], awp[:, 0:2], AF.Exp)
        ssp = ppq.tile([128, 1024], F32, tag='q')
        nc.tensor.matmul(ssp[0:2, 0:1], aw_e[:], ones_f[:], start=True, stop=True,
                         skip_group_check=True)
        ssb = ph.tile([2, 1], F32)
        nc.vector.tensor_copy(ssb[:], ssp[0:2, 0:1])
        stp = ppq.tile([128, 1024], F32, tag='q')
        nc.tensor.transpose(stp[0:1, 0:2], ssb[0:2, 0:1], i1f[0:2, 0:2])
        tot = ph.tile([1, 4], F32)
        nc.vector.tensor_copy(tot[0:1, 2:4], stp[0:1, 0:2])
        nc.vector.tensor_tensor(tot[0:1, 0:1], tot[0:1, 2:3], tot[0:1, 3:4], op=OP.add)
        nc.vector.reciprocal_approx_fast(tot[0:1, 1:2], tot[0:1, 0:1])
        rb2p = ppq.tile([128, 1024], F32, tag='q', name="rb2p")
        nc.tensor.matmul(rb2p[:, 0:1], ones_r[0:1, :], tot[0:1, 1:2],
                         start=True, stop=True, skip_group_check=True)
        rb2 = ph.tile([128, 1], F32)
        nc.vector.tensor_copy(rb2[:], rb2p[:, 0:1])
        aw_nb = ph.tile([128, 2], BF16)
        nc.vector.tensor_scalar(aw_nb[:], aw_e[:], rb2[:, 0:1], None, OP.mult)
        # ccT [2048, 1] packed as [128, 16]
        ccp = ppq.tile([128, 1024], F32, tag='q')
        for u16 in range(16):
            for m2 in range(2):
                nc.tensor.matmul(ccp[:, u16:u16 + 1],
                                 eh[:, C * m2 + 128 * u16:C * m2 + 128 * u16 + 128],
                                 aw_nb[:, m2:m2 + 1],
                                 start=(u16 == 0 and m2 == 0),
                                 stop=(u16 == 15 and m2 == 1),
                                 skip_group_check=True)
        ccb = ph.tile([128, 16], BF16)
        nc.vector.tensor_copy(ccb[:], ccp[:, 0:16])
        # z = cc @ kg_W1  [1, 1024]
        zp2 = ppc.tile([128, 1024], F32, tag='c')
        for u in range(16):
            rhs = pks.tile([128, 512], BF16, tag="k1c")
            rhs2 = pks.tile([128, 512], BF16, tag="k1c")
            nc.sync.dma_start(rhs[:], k1_d[128 * u:128 * u + 128, 0:512])
            nc.sync.dma_start(rhs2[:], k1_d[128 * u:128 * u + 128, 512:1024])
            nc.tensor.matmul(zp2[0:1, 0:512], ccb[:, u:u + 1], rhs[:],
                             start=(u == 0), stop=(u == 15), skip_group_check=True)
            nc.tensor.matmul(zp2[0:1, 512:1024], ccb[:, u:u + 1], rhs2[:],
                             start=(u == 0), stop=(u == 15), skip_group_check=True)
        # LN + gelu on [1, 1024]  (rows of one packed tile: 0=z, 1=xc, 2=gelu)
        zx3 = ph.tile([1, 2 * D], F32)
        zgf = ph.tile([1, D], F32)
        zst = ph.tile([1, 8], F32)
        nc.vector.tensor_copy(zx3[0:1, 0:D], zp2[0:1, 0:1024])
        nc.vector.tensor_reduce(zst[0:1, 0:1], zx3[0:1, 0:D], mybir.AxisListType.X, OP.add)
        nc.vector.tensor_scalar(zst[0:1, 1:2], zst[0:1, 0:1], -1.0 / D, None, OP.mult)
        nc.scalar.activation(zx3[0:1, D:2 * D], zx3[0:1, 0:D], AF.Identity, bias=zst[0:1, 1:2])
        nc.scalar.activation(zx3[0:1, 0:D], zx3[0:1, D:2 * D], AF.Square, accum_out=zst[0:1, 2:3])
        nc.vector.tensor_scalar(zst[0:1, 3:4], zst[0:1, 2:3], 1.0 / D, None, OP.mult)
        nc.scalar.activation(zst[0:1, 4:5], zst[0:1, 3:4], AF.Ln, bias=eps_t[0:1, 0:1])
        nc.scalar.activation(zst[0:1, 5:6], zst[0:1, 4:5], AF.Exp, scale=-0.5)
        nc.scalar.activation(zgf[0:1, :], zx3[0:1, D:2 * D], AF.Gelu, scale=zst[0:1, 5:6])
        # zgT [1024,1] packed [128, 8] via PE transposes
        ztp = ppq.tile([128, 1024], F32, tag='q')
        for k in range(8):
            nc.tensor.transpose(ztp[:, k:k + 1], zgf[0:1, 128 * k:128 * k + 128],
                                i1f[0:1, 0:1])
        zgt = ph.tile([128, 8], BF16)
        nc.vector.tensor_copy(zgt[:], ztp[:, 0:8])
        # kp [1, 72]
        kpp = ppc.tile([128, 1024], F32, tag='c')
        for k in range(8):
            nc.tensor.matmul(kpp[0:1, 0:72], zgt[:, k:k + 1],
                             k2[:, 72 * k:72 * k + 72],
                             start=(k == 0), stop=(k == 7), skip_group_check=True)
        kpe = ph.tile([1, 72], F32)
        nc.scalar.activation(kpe[:], kpp[0:1, 0:72], AF.Exp)
        ksum = ph.tile([1, 16], F32)
        nc.vector.tensor_reduce(ksum[0:1, 0:8],
                                kpe[:].rearrange("p (h w) -> p h w", h=8),
                                mybir.AxisListType.X, OP.add)
        nc.vector.reciprocal_approx_fast(ksum[0:1, 8:16], ksum[0:1, 0:8])
        kpn = ph.tile([1, 72], F32)
        for j in range(8):
            nc.vector.tensor_scalar(kpn[0:1, 9 * j:9 * j + 9],
                                    kpe[0:1, 9 * j:9 * j + 9],
                                    ksum[0:1, 8 + j:9 + j], None, OP.mult)
        wvp = ppq.tile([128, 1024], F32, tag='q', name="wvp")
        nc.tensor.matmul(wvp[:, 0:72], ones_r[0:1, :], kpn[0:1, :],
                         start=True, stop=True, skip_group_check=True)
        wvec = ph.tile([128, 72], F32)
        nc.vector.tensor_copy(wvec[:], wvp[:, 0:72])

        # ---------------- qkv projections ----------------
        qt = pwp.tile([128, 4 * T], BF16)
        ktp = pwp.tile([128, 4 * KTP], BF16)
        nc.vector.memset(ktp[:], 0.0)
        for t in range(4):
            pq = ppq.tile([128, 1024], F32, tag='q')
            pk = ppc.tile([128, 1024], F32, tag='c')
            for n in range(2):
                for k in range(8):
                    nc.tensor.matmul(pq[:, 512 * n:512 * n + 512],
                                     wq[:, 512 * k + 128 * t:512 * k + 128 * t + 128],
                                     xt[:, T * k + 512 * n:T * k + 512 * n + 512],
                                     start=(k == 0), stop=(k == 7),
                                     skip_group_check=True)
                    nc.tensor.matmul(pk[:, 512 * n:512 * n + 512],
                                     wk[:, 512 * k + 128 * t:512 * k + 128 * t + 128],
                                     xt[:, T * k + 512 * n:T * k + 512 * n + 512],
                                     start=(k == 0), stop=(k == 7),
                                     skip_group_check=True)
            nc.scalar.copy(qt[:, T * t:T * t + T], pq[:])
            nc.scalar.copy(ktp[:, KTP * t + 1:KTP * t + 1 + T], pk[:])
        vpp = pwp.tile([128, NCH * 520], BF16)
        nc.vector.memset(vpp[:], 1.0)
        for c in range(NCH):
            M_c = _chunk_rows(c)
            pv_ = ppc.tile([128, 1024], F32, tag='c')
            for k in range(8):
                nc.tensor.matmul(pv_[0:M_c, 0:512],
                                 xt[:, T * k + CH * c:T * k + CH * c + M_c],
                                 wv[:, 512 * k:512 * k + 512],
                                 start=(k == 0), stop=(k == 7),
                                 skip_group_check=True)
            nc.vector.tensor_copy(
                vpp[0:M_c, 520 * c:520 * c + 520]
                    .rearrange("p (h w) -> p h w", h=8)[:, :, 0:64],
                pv_[0:M_c, 0:512].rearrange("p (h w) -> p h w", h=8))

        if DBG:
            nc.sync.dma_start(dbg["dqt"][:], qt[:])
            nc.sync.dma_start(dbg["dktp"][:], ktp[:])
            nc.sync.dma_start(dbg["deh"][:], eh[:])
            nc.sync.dma_start(dbg["dkpn"][:], kpn[:])
            nc.sync.dma_start(dbg["dvpp"][:], vpp[:])
        # ---------------- attention maps ----------------
        oT = pwp.tile([128, 4 * T], BF16)
        for j in range(8):
            t, prow = j // 2, 64 * (j % 2)
            # banded [128, 3*128] for this map
            bd = pbd.tile([128, 3 * 128], BF16)
            for dq in range(3):
                nc.vector.tensor_scalar(
                    bd[:, 128 * dq:128 * dq + 128], i3[:, 0:128],
                    wvec[:, 9 * j + 3 * dq:9 * j + 3 * dq + 1], None, OP.mult)
                for d in (1, 2):
                    nc.vector.scalar_tensor_tensor(
                        bd[:, 128 * dq:128 * dq + 128],
                        i3[:, 128 * d:128 * d + 128],
                        wvec[:, 9 * j + 3 * dq + d:9 * j + 3 * dq + d + 1],
                        bd[:, 128 * dq:128 * dq + 128],
                        op0=OP.mult, op1=OP.add)
            ovp = ppv.tile([65, 1024], F32, tag='v')
            for c in range(NCH):
                M_c, Qs = _chunk_rows(c), QSTART[c]
                lo_all = QW * Qs
                # qk into per-half psum tiles
                halves = {}
                for h2 in (0, 1):
                    h_lo, h_hi = 512 * h2, 512 * h2 + 512
                    lo = max(lo_all, h_lo)
                    if lo >= h_hi:
                        continue
                    qh = ppk.tile([128, 512], F32, tag='k', name="qkp")
                    halves[h2] = (qh, h_lo)
                    nc.tensor.matmul(qh[:, lo - h_lo:512],
                                     ktp[64 * (j % 2):64 * (j % 2) + 64,
                                         KTP * t + CH * c:KTP * t + CH * c + 128],
                                     qt[64 * (j % 2):64 * (j % 2) + 64,
                                        T * t + lo:T * t + h_hi],
                                     start=True, stop=True, skip_group_check=True)
                # Z with causal zero-mask
                z_ = pz.tile([128, 1024], BF16)
                if Qs > 0:
                    nc.vector.memset(z_[:, lo_all - 1:lo_all], 0.0)
                runs = []
                for Q in range(Qs, 4):
                    sl = (QW * Q, QW * Q + QW)
                    qh, h_lo = halves[Q // 2]
                    if (c, Q) in MASK_PAIRS:
                        n = MASK_PAIRS.index((c, Q))
                        nc.vector.tensor_tensor(
                            z_[:, sl[0]:sl[1]],
                            qh[:, sl[0] - h_lo:sl[1] - h_lo],
                            mq[:, QW * n:QW * n + QW], op=OP.mult)
                    else:
                        if runs and runs[-1][1] == sl[0] and (runs[-1][0] // 512) == (sl[0] // 512):
                            runs[-1] = (runs[-1][0], sl[1])
                        else:
                            runs.append(sl)
                for (r0, r1) in runs:
                    qh, h_lo = halves[r0 // 512]
                    nc.scalar.copy(z_[:, r0:r1], qh[:, r0 - h_lo:r1 - h_lo])
                if DBG and j == 0 and c == 0:
                    nc.sync.dma_start(dbg["dz0"][:], z_[:])
                # conv: banded matmuls grouped by lhsT (ldweights reuse)
                cvp = ppc.tile([128, 1024], F32, tag='c')
                first_in_bank = {0: True, 1: True}
                mm_list = []
                for dq in range(3):
                    sh = dq - 1
                    for Q in range(Qs, 4):
                        lo, hi = QW * Q, QW * Q + QW
                        rl, rh, ol = lo + sh, hi + sh, lo
                        if rl < 0:
                            rl, ol = 0, lo + 1
                        if rh > T:
                            rh = T
                        mm_list.append((Q, bd[:, 128 * dq:128 * dq + M_c],
                                        z_[:, rl:rh], (ol, ol + rh - rl)))
                for Q in range(Qs, 4):
                    if (c, Q) in MBIG_PAIRS:
                        n = MBIG_PAIRS.index((c, Q))
                        mm_list.append((Q, i3[:, 0:M_c],
                                        mb[:, QW * n:QW * n + QW],
                                        (QW * Q, QW * Q + QW)))
                last_in_bank = {}
                for idx, (Q, _, _, _) in enumerate(mm_list):
                    last_in_bank[Q // 2] = idx
                for idx, (Q, lhsT, rhs, (ol, oh)) in enumerate(mm_list):
                    bank = Q // 2
                    nc.tensor.matmul(cvp[0:M_c, ol:oh], lhsT, rhs,
                                     start=first_in_bank[bank],
                                     stop=(last_in_bank[bank] == idx),
                                     skip_group_check=True)
                    first_in_bank[bank] = False
                # exp -> E
                e_ = pe_.tile([128, 1024], BF16)
                nc.scalar.activation(e_[0:M_c, lo_all:T], cvp[0:M_c, lo_all:T],
                                     AF.Exp)
                if DBG and j == 0 and c == 0:
                    nc.sync.dma_start(dbg["de0"][:], e_[:])
                # PV accumulate (ones-column gives softmax sums in row 64)
                for Q in range(Qs, 4):
                    lo, hi = QW * Q, QW * Q + QW
                    nc.tensor.matmul(ovp[0:65, lo:hi],
                                     vpp[0:M_c, 520 * c + 65 * j:520 * c + 65 * j + 65],
                                     e_[0:M_c, lo:hi],
                                     start=(c == 0 and Q % 2 == 0),
                                     stop=(c == NCH - 1 and Q % 2 == 1),
                                     skip_group_check=True)
            if DBG and j == 0:
                ovsb = ppt.tile([128, 1024], F32, name="ovsbdbg")
                nc.vector.tensor_copy(ovsb[0:65, :], ovp[0:65, 0:T])
                nc.sync.dma_start(dbg["dov"][:], ovsb[0:65, :])
            # normalize -> oT
            ssb = pn.tile([1, 2 * T], F32, name="ssb")
            nc.scalar.copy(ssb[0:1, 0:T], ovp[64:65, 0:T])
            nc.vector.reciprocal_approx_fast(ssb[0:1, T:2 * T], ssb[0:1, 0:T])
            rbp = ppq.tile([128, 1024], F32, tag='q', name="rbp")
            nc.tensor.matmul(rbp[0:64, 0:512], ones_r[0:1, 0:64],
                             ssb[0:1, T:T + 512], start=True, stop=True,
                             skip_group_check=True)
            nc.tensor.matmul(rbp[0:64, 512:1024], ones_r[0:1, 0:64],
                             ssb[0:1, T + 512:2 * T], start=True, stop=True,
                             skip_group_check=True)
            rb = pn.tile([64, T], F32)
            nc.scalar.copy(rb[:], rbp[0:64, 0:1024])
            nc.vector.tensor_tensor(oT[prow:prow + 64, T * t:T * t + T],
                                    ovp[0:64, 0:T], rb[:], op=OP.mult)

        # ---------------- output projection partials ----------------
        for m in range(8):
            pp_ = ppq.tile([128, 1024], F32, tag='q')
            for n in range(2):
                for k in range(4):
                    nc.tensor.matmul(pp_[:, 512 * n:512 * n + 512],
                                     oT[:, T * k + 128 * m:T * k + 128 * m + 128],
                                     pw[:, D * k + 512 * n:D * k + 512 * n + 512],
                                     start=(k == 0), stop=(k == 3),
                                     skip_group_check=True)
            psb = ppt.tile([128, 1024], BF16)
            nc.scalar.copy(psb[:], pp_[:])
            nc.sync.dma_start(part_d[128 * m:128 * m + 128, :], psb[:])

        if DBG:
            nc.sync.dma_start(dbg["doT"][:], oT[:])
            nc.sync.dma_start(dbg["dpart"][:], part_d[:])
        # ---------------- pair ReduceScatter + output ----------------
        cc_sem = nc.alloc_semaphore("cc_sem")
        od_sem = nc.alloc_semaphore("od_sem")
        with tc.tile_critical():
            nc.gpsimd.collective_compute(
                "ReduceScatter", OP.add,
                replica_groups=[[0, 1], [2, 3], [4, 5], [6, 7]],
                ins=[part_d[:]], outs=[crs_d[:]],
            ).then_inc(cc_sem)
            nc.gpsimd.wait_ge(cc_sem, 1)
            nc.gpsimd.dma_start(out_d[:], crs_d[:]).then_inc(od_sem, 16)
            nc.gpsimd.wait_ge(od_sem, 16)

    nc.compile()
    _CACHE["nc"] = nc
    return nc


def _host_inputs(inputs):
    i3, i1f, maskq, mbig = _host_consts()
    x = np.asarray(inputs["x"], np.float32)
    hist = np.asarray(inputs["historical_data"], np.float32)
    g = lambda n: np.asarray(inputs[n], np.float32)
    Wq, Wk, Wv = g("Wq"), g("Wk"), g("Wv")
    pW = g("proj_W"); hW = g("hist_W"); c1 = g("ctx_W1"); c2 = g("ctx_W2")
    k1 = g("kg_W1"); k2 = g("kg_W2")
    in_maps = []
    for core in range(N_CORES):
        b, H2 = core // 2, core % 2
        cols = slice(512 * H2, 512 * H2 + 512)
        in_maps.append({
            "xt": np.ascontiguousarray(x[b].T).astype(bf16),
            "wq": (Wq[:, cols] / 8.0).astype(bf16),
            "wk": Wk[:, cols].astype(bf16),
            "wv": Wv[:, cols].astype(bf16),
            "pw": pW[cols, :].astype(bf16),
            "ht": np.ascontiguousarray(hist[b].T).astype(bf16),
            "hw": hW.astype(bf16),
            "c1": c1.astype(bf16),
            "c2": c2.astype(bf16),
            "k1": k1.astype(bf16),
            "k2": k2[:, 72 * H2:72 * H2 + 72].astype(bf16),
            "i3": i3, "i1f": i1f, "mq": maskq, "mb": mbig,
        })
    return in_maps


def run(inputs, trace=False):
    nc = _build()
    in_maps = _host_inputs(inputs)
    res = run_bass_kernel_spmd(nc, in_maps, list(range(N_CORES)), trace=trace)
    out = np.empty((B, T, D), np.float32)
    for b in range(B):
        out[b, 0:512] = res.results[2 * b]["out"]
        out[b, 512:1024] = res.results[2 * b + 1]["out"]
    return out, res


def kernel(**inputs):
    out, _ = run(inputs, trace=False)
    return out


# revision 32
# speedup vs baseline: 1.4009x; 1.4009x over previous
"""Causal adaptive-kernel attention on 8 TRN2 NeuronCores (Bass/Tile).

Sharding: core i handles batch b = i//2 and heads 8*(i%2) .. 8*(i%2)+8
(d columns 512*(i%2) .. +512).  The per-(b,h) attention maps are computed
in a TRANSPOSED layout S^T[kj, qi] so that:
  - qk^T, the 3x3 conv (as 3 banded matmuls over the kj axis, with the
    qi shifts free via access-pattern offsets), the -1e9 causal mask add,
    and P@V all run on the TensorEngine,
  - the softmax denominators come for free from a ones-column appended
    to V (row 64 of the PV accumulator),
  - the output lands pre-transposed as o^T = the exact lhsT layout the
    final projection needs.
The per-(b,h) 3x3 kernels are generated on-device (hist->context->kernel
MLP, layernorm/gelu/softmax on ACT+DVE) and turned into banded matrices
via scalar_tensor_tensor against constant shifted-identity masks.
The two cores sharing a batch pair-ReduceScatter their projection
partials; the host concatenates the row halves.
"""
import numpy as np
import ml_dtypes
from contextlib import ExitStack

import os
import concourse.bass as bass
import concourse.bacc as bacc
import concourse.tile as tile
from concourse import mybir
from concourse.bass_utils import run_bass_kernel_spmd

F32 = mybir.dt.float32
BF16 = mybir.dt.bfloat16
AF = mybir.ActivationFunctionType
OP = mybir.AluOpType

B, T, D, H, hd, Th, C = 4, 1024, 1024, 16, 64, 256, 2048
CH, NCH, QW = 126, 9, 256            # conv chunk stride, #chunks, quarter width
N_CORES = 8
KTP = 1136                            # padded kT row length (col = tok+1)

bf16 = ml_dtypes.bfloat16


def _chunk_rows(c):
    return min(CH, T - CH * c)


def _tables():
    mask_pairs, mbig_pairs, qstart = [], [], {}
    for c in range(NCH):
        M_c = _chunk_rows(c)
        qs = None
        for Q in range(4):
            qi = np.arange(QW * Q, QW * Q + QW)
            kj_out = CH * c + np.arange(M_c)
            msk = qi[None, :] < kj_out[:, None]
            if msk.all():
                continue
            if qs is None:
                qs = Q
            kj_z = CH * c - 1 + np.arange(128)
            if (qi[None, :] < kj_z[:, None]).any():
                mask_pairs.append((c, Q))
            if msk.any():
                mbig_pairs.append((c, Q))
        qstart[c] = qs
    return mask_pairs, mbig_pairs, qstart


MASK_PAIRS, MBIG_PAIRS, QSTART = _tables()
NMQ = len(MASK_PAIRS)


def _host_consts():
    i3 = np.zeros((128, 3 * 128), np.float32)
    for d in range(3):
        idx = np.arange(128 - d)
        i3[idx + d, 128 * d + idx] = 1.0
    i1f = np.eye(128, dtype=np.float32)
    maskq = np.zeros((128, NMQ * QW), np.float32)
    for n, (c, Q) in enumerate(MASK_PAIRS):
        kj_z = CH * c - 1 + np.arange(128)
        qi = np.arange(QW * Q, QW * Q + QW)
        maskq[:, n * QW:(n + 1) * QW] = (qi[None, :] >= kj_z[:, None])
    mbig = np.zeros((128, NMQ * QW), np.float32)
    for n, (c, Q) in enumerate(MBIG_PAIRS):
        kj_out = CH * c + np.arange(128)      # rows >= M_c unused
        qi = np.arange(QW * Q, QW * Q + QW)
        mbig[:, n * QW:(n + 1) * QW] = np.where(
            qi[None, :] < kj_out[:, None], -1e9, 0.0)
    return i3.astype(bf16), i1f, maskq.astype(bf16), mbig.astype(bf16)


_CACHE = {}


def _build():
    if "nc" in _CACHE:
        return _CACHE["nc"]
    nc = bacc.Bacc("TRN2", target_bir_lowering=False, debug=False,
                   num_devices=N_CORES)

    def din(name, shape, dt=BF16):
        return nc.dram_tensor(name, shape, dt, kind="ExternalInput").ap()

    xt_d = din("xt", [D, T])                      # x[b].T
    wq_d = din("wq", [D, 512])                    # Wq[:, cols] / 8
    wk_d = din("wk", [D, 512])
    wv_d = din("wv", [D, 512])
    pw_d = din("pw", [512, D])                    # proj_W[cols, :]
    ht_d = din("ht", [D, Th])                     # hist[b].T
    hw_d = din("hw", [D, C])                      # hist_W
    c1_d = din("c1", [C, 512])                    # ctx_W1
    c2_d = din("c2", [512, 1])                    # ctx_W2
    k1_d = din("k1", [C, D])                      # kg_W1
    k2_d = din("k2", [D, 72])                     # kg_W2[:, my 72]
    i3_d = din("i3", [128, 3 * 128])
    i1f_d = din("i1f", [128, 128], F32)
    mq_d = din("mq", [128, NMQ * QW])
    mb_d = din("mb", [128, NMQ * QW])

    out_d = nc.dram_tensor("out", [512, D], F32, kind="ExternalOutput").ap()
    DBG = bool(os.environ.get("BASSDBG"))
    dbg = {}
    if DBG:
        for nm, shape, dt in [("dqt", [128, 4 * T], BF16), ("dktp", [128, 4 * KTP], BF16),
                              ("deh", [128, 2 * C], BF16), ("dkpn", [1, 72], F32),
                              ("doT", [128, 4 * T], BF16), ("dvpp", [128, NCH * 520], BF16),
                              ("dz0", [128, T], BF16), ("de0", [128, T], BF16),
                              ("dov", [65, T], F32), ("dpart", [T, D], BF16)]:
            dbg[nm] = nc.dram_tensor(nm, shape, dt, kind="ExternalOutput").ap()
    part_d = nc.dram_tensor("part", [512, D], BF16)
    part2_d = nc.dram_tensor("part2", [512, D], BF16)
    crs_d = nc.dram_tensor("crs", [512, D], BF16)

    with tile.TileContext(nc) as tc, ExitStack() as ctx:
        pc = ctx.enter_context(tc.tile_pool(name="pc", bufs=1))
        pwp = ctx.enter_context(tc.tile_pool(name="pwp", bufs=1))
        ph = ctx.enter_context(tc.tile_pool(name="ph", bufs=1))
        pks = ctx.enter_context(tc.tile_pool(name="pks", bufs=4))
        pz = ctx.enter_context(tc.tile_pool(name="pz", bufs=4))
        pe_ = ctx.enter_context(tc.tile_pool(name="pe", bufs=2))
        pbd = ctx.enter_context(tc.tile_pool(name="pbd", bufs=8))
        pn = ctx.enter_context(tc.tile_pool(name="pn", bufs=2))
        ppt = ctx.enter_context(tc.tile_pool(name="ppt", bufs=1))
        ppk = ctx.enter_context(tc.tile_pool(name="ppk", bufs=2, space="PSUM"))
        ppc = ctx.enter_context(tc.tile_pool(name="ppc", bufs=2, space="PSUM"))
        ppv = ctx.enter_context(tc.tile_pool(name="ppv", bufs=1, space="PSUM"))

        # ---------------- constant / weight loads ----------------
        i3 = pc.tile([128, 3 * 128], BF16); nc.sync.dma_start(i3[:], i3_d[:])
        i1f = pc.tile([128, 128], F32); nc.sync.dma_start(i1f[:], i1f_d[:])
        mq = pc.tile([128, NMQ * QW], BF16)
        mb = pc.tile([128, NMQ * QW], BF16)
        ones_f = pc.tile([128, 1], F32); nc.vector.memset(ones_f[:], 1.0)
        ones_r = pc.tile([1, 128], F32); nc.vector.memset(ones_r[:], 1.0)
        eps_t = pc.tile([128, 1], F32); nc.vector.memset(eps_t[:], 1e-5)

        def load_kp(dram, kchunks, m, nm):
            t = pwp.tile([128, kchunks * m], BF16, name=nm)
            nc.sync.dma_start(
                t[:].rearrange("p (k m) -> p k m", k=kchunks),
                dram.rearrange("(k p) m -> p k m", p=128))
            return t

        ht = load_kp(ht_d, 8, Th, "ht_t")         # [128, 8*256]
        k2 = load_kp(k2_d, 8, 72, "k2_t")
        c2 = pwp.tile([128, 4], BF16)
        nc.sync.dma_start(c2[:].rearrange("p (k m) -> p k m", k=4),
                          c2_d.rearrange("(k p) m -> p k m", p=128))

        # ---------------- kg path: eh = gelu(ln(hist @ hist_W)) ----------------
        eh = ph.tile([128, 2 * C], BF16)           # tok chunk m at cols 2048m
        ehT = ph.tile([128, 16 * Th], BF16)        # C chunk k at cols 256k
        zeh = ph.tile([128, 2 * C], F32)           # both tok chunks
        stats = ph.tile([128, 16], F32)
        rstd2 = ph.tile([128, 2], F32)
        xc = ph.tile([128, C], F32)
        zps = {}
        for half in range(2):                      # psum rounds: cols n=2*half..
            for m in range(2):
                zps[m] = ppc.tile([128, 1024], F32, tag='c', name=f"zp{m}")
            for n2 in range(2):
                n = 2 * half + n2
                for k in range(8):
                    rhs = pks.tile([128, 512], BF16, tag="hwc")
                    nc.sync.dma_start(rhs[:], hw_d[128 * k:128 * k + 128,
                                                   512 * n:512 * n + 512])
                    for m in range(2):
                        nc.tensor.matmul(zps[m][:, 512 * n2:512 * n2 + 512],
                                         ht[:, 256 * k + 128 * m:256 * k + 128 * m + 128],
                                         rhs[:], start=(k == 0), stop=(k == 7),
                                         skip_group_check=True)
                for m in range(2):
                    nc.scalar.activation(
                        zeh[:, C * m + 512 * n:C * m + 512 * n + 512],
                        zps[m][:, 512 * n2:512 * n2 + 512], AF.Copy,
                        accum_out=stats[:, 8 * m + n:8 * m + n + 1])
        for m in range(2):
            nc.vector.tensor_reduce(stats[:, 8 * m + 4:8 * m + 5],
                                    stats[:, 8 * m:8 * m + 4],
                                    mybir.AxisListType.X, OP.add)
            nc.vector.tensor_scalar(stats[:, 8 * m + 5:8 * m + 6],
                                    stats[:, 8 * m + 4:8 * m + 5],
                                    -1.0 / C, None, OP.mult)
            nc.scalar.activation(xc[:], zeh[:, C * m:C * m + C], AF.Identity,
                                 bias=stats[:, 8 * m + 5:8 * m + 6])
            nc.scalar.activation(zeh[:, C * m:C * m + C], xc[:], AF.Square,
                                 accum_out=stats[:, 8 * m + 6:8 * m + 7])
            nc.vector.tensor_scalar(stats[:, 8 * m + 7:8 * m + 8],
                                    stats[:, 8 * m + 6:8 * m + 7],
                                    1.0 / C, None, OP.mult)
            nc.scalar.activation(stats[:, 8 * m:8 * m + 1],
                                 stats[:, 8 * m + 7:8 * m + 8],
                                 AF.Ln, bias=eps_t[:, 0:1])
            nc.scalar.activation(rstd2[:, m:m + 1], stats[:, 8 * m:8 * m + 1],
                                 AF.Exp, scale=-0.5)
            nc.scalar.activation(eh[:, C * m:C * m + C], xc[:], AF.Gelu,
                                 scale=rstd2[:, m:m + 1])
        xt = load_kp(xt_d, 8, T, "xt_t")          # [128, 8*1024]
        wq = load_kp(wq_d, 8, 512, "wq_t")
        wk = load_kp(wk_d, 8, 512, "wk_t")
        wv = load_kp(wv_d, 8, 512, "wv_t")
        # ehT via PE transposes (4 k-blocks per psum tile, strided ACT copy out)
        for g in range(4):
            for m in range(2):
                tps = ppk.tile([128, 512], BF16, tag="k", name="tps")
                for k2_ in range(4):
                    k = 4 * g + k2_
                    nc.tensor.transpose(tps[:, 128 * k2_:128 * k2_ + 128],
                                        eh[:, C * m + 128 * k:C * m + 128 * k + 128],
                                        i3[:, 0:128])
                nc.scalar.copy(
                    ehT[:].rearrange("p (k m w) -> p k m w", k=16, m=2)
                       [:, 4 * g:4 * g + 4, m, :],
                    tps[:].rearrange("p (k w) -> p k w", k=4))
        # g1T = ctx_W1^T @ ehT   [512(4 chunks), 256]
        g1gT = ph.tile([128, 4 * Th], BF16)
        g1p = ppc.tile([128, 1024], F32, tag='c')
        for k in range(16):
            c1c = pks.tile([128, 512], BF16, tag="hwc")
            nc.sync.dma_start(c1c[:], c1_d[128 * k:128 * k + 128, :])
            for mj in range(4):
                nc.tensor.matmul(g1p[:, 256 * mj:256 * mj + 256],
                                 c1c[:, 128 * mj:128 * mj + 128],
                                 ehT[:, 256 * k:256 * k + 256],
                                 start=(k == 0 and mj % 2 == 0),
                                 stop=(k == 15 and mj % 2 == 1),
                                 skip_group_check=True)
        nc.scalar.activation(g1gT[:], g1p[:], AF.Gelu)
        # aw logits [256, 1] -> softmax over Th
        awp = ppc.tile([128, 1024], F32, tag='c')
        for m2 in range(2):
            for u in range(4):
                nc.tensor.matmul(awp[:, m2:m2 + 1],
                                 g1gT[:, 256 * u + 128 * m2:256 * u + 128 * m2 + 128],
                                 c2[:, u:u + 1],
                                 start=(u == 0 and m2 == 0), stop=(u == 3 and m2 == 1),
                                 skip_group_check=True)
        aw_e = ph.tile([128, 2], F32)
        nc.scalar.activation(aw_e[:], awp[:, 0:2], AF.Exp)
        ssp = ppk.tile([128, 512], F32, tag='k', name='ssp')
        nc.tensor.matmul(ssp[0:2, 0:1], aw_e[:], ones_f[:], start=True, stop=True,
                         skip_group_check=True)
        ssb = ph.tile([2, 1], F32)
        nc.vector.tensor_copy(ssb[:], ssp[0:2, 0:1])
        stp = ppk.tile([128, 512], F32, tag='k', name='stp')
        nc.tensor.transpose(stp[0:1, 0:2], ssb[0:2, 0:1], i1f[0:2, 0:2])
        tot = ph.tile([1, 4], F32)
        nc.vector.tensor_copy(tot[0:1, 2:4], stp[0:1, 0:2])
        nc.vector.tensor_tensor(tot[0:1, 0:1], tot[0:1, 2:3], tot[0:1, 3:4], op=OP.add)
        nc.vector.reciprocal_approx_fast(tot[0:1, 1:2], tot[0:1, 0:1])
        rb2p = ppk.tile([128, 512], F32, tag='k', name="rb2p")
        nc.tensor.matmul(rb2p[:, 0:1], ones_r[0:1, :], tot[0:1, 1:2],
                         start=True, stop=True, skip_group_check=True)
        rb2 = ph.tile([128, 1], F32)
        nc.vector.tensor_copy(rb2[:], rb2p[:, 0:1])
        aw_nb = ph.tile([128, 2], BF16)
        nc.vector.tensor_scalar(aw_nb[:], aw_e[:], rb2[:, 0:1], None, OP.mult)
        # ccT [2048, 1] packed as [128, 16]
        ccp = ppk.tile([128, 512], F32, tag='k', name='ccp')
        for u16 in range(16):
            for m2 in range(2):
                nc.tensor.matmul(ccp[:, u16:u16 + 1],
                                 eh[:, C * m2 + 128 * u16:C * m2 + 128 * u16 + 128],
                                 aw_nb[:, m2:m2 + 1],
                                 start=(u16 == 0 and m2 == 0),
                                 stop=(u16 == 15 and m2 == 1),
                                 skip_group_check=True)
        ccb = ph.tile([128, 16], BF16)
        nc.vector.tensor_copy(ccb[:], ccp[:, 0:16])
        # z = cc @ kg_W1  [1, 1024]
        zp2 = ppc.tile([128, 1024], F32, tag='c')
        for u in range(16):
            rhs = pks.tile([128, 512], BF16, tag="k1c")
            rhs2 = pks.tile([128, 512], BF16, tag="k1c")
            nc.sync.dma_start(rhs[:], k1_d[128 * u:128 * u + 128, 0:512])
            nc.sync.dma_start(rhs2[:], k1_d[128 * u:128 * u + 128, 512:1024])
            nc.tensor.matmul(zp2[0:1, 0:512], ccb[:, u:u + 1], rhs[:],
                             start=(u == 0), stop=(u == 15), skip_group_check=True)
            nc.tensor.matmul(zp2[0:1, 512:1024], ccb[:, u:u + 1], rhs2[:],
                             start=(u == 0), stop=(u == 15), skip_group_check=True)
        # LN + gelu on z in transposed [128, 8] layout (partition-parallel)
        zsb = ph.tile([1, D], F32)
        nc.scalar.copy(zsb[:], zp2[0:1, 0:1024])
        ztf = ppk.tile([128, 512], F32, tag='k', name="ztf")
        for k in range(8):
            nc.tensor.transpose(ztf[:, k:k + 1], zsb[0:1, 128 * k:128 * k + 128],
                                i1f[0:1, 0:1])
        zt = ph.tile([128, 8], F32)
        nc.vector.tensor_copy(zt[:], ztf[:, 0:8])
        zst2 = ph.tile([128, 12], F32)
        zsc = ph.tile([1, 16], F32)
        nc.vector.tensor_reduce(zst2[:, 0:1], zt[:], mybir.AxisListType.X, OP.add)
        nc.scalar.activation(zst2[:, 4:12], zt[:], AF.Square,
                             accum_out=zst2[:, 1:2])
        zsp = ppk.tile([128, 512], F32, tag='k', name="zsp")
        nc.tensor.matmul(zsp[0:1, 0:1], zst2[:, 0:1], ones_f[:],
                         start=True, stop=True, skip_group_check=True)
        nc.tensor.matmul(zsp[0:1, 1:2], zst2[:, 1:2], ones_f[:],
                         start=True, stop=True, skip_group_check=True)
        nc.vector.tensor_scalar(zsc[0:1, 0:2], zsp[0:1, 0:2], 1.0 / D, None, OP.mult)
        nc.vector.tensor_scalar(zsc[0:1, 2:3], zsc[0:1, 0:1], zsc[0:1, 0:1], None, OP.mult)
        nc.vector.tensor_tensor(zsc[0:1, 3:4], zsc[0:1, 1:2], zsc[0:1, 2:3], op=OP.subtract)
        nc.scalar.activation(zsc[0:1, 6:7], zsc[0:1, 3:4], AF.Ln, bias=eps_t[0:1, 0:1])
        nc.scalar.activation(zsc[0:1, 4:5], zsc[0:1, 6:7], AF.Exp, scale=-0.5)
        nc.vector.tensor_scalar(zsc[0:1, 7:8], zsc[0:1, 0:1], zsc[0:1, 4:5], None, OP.mult)
        nc.vector.tensor_scalar(zsc[0:1, 5:6], zsc[0:1, 7:8], -1.0, None, OP.mult)
        zbp = ppk.tile([128, 512], F32, tag='k', name="zbp")
        nc.tensor.matmul(zbp[:, 0:2], ones_r[0:1, :], zsc[0:1, 4:6],
                         start=True, stop=True, skip_group_check=True)
        zbc = ph.tile([128, 2], F32)
        nc.vector.tensor_copy(zbc[:], zbp[:, 0:2])
        zgt = ph.tile([128, 8], BF16)
        nc.scalar.activation(zgt[:], zt[:], AF.Gelu,
                             scale=zbc[:, 0:1], bias=zbc[:, 1:2])
        # kp [1, 72]
        kpp = ppc.tile([128, 1024], F32, tag='c')
        for k in range(8):
            nc.tensor.matmul(kpp[0:1, 0:72], zgt[:, k:k + 1],
                             k2[:, 72 * k:72 * k + 72],
                             start=(k == 0), stop=(k == 7), skip_group_check=True)
        kpe = ph.tile([1, 72], F32)
        nc.scalar.activation(kpe[:], kpp[0:1, 0:72], AF.Exp)
        ksum = ph.tile([1, 16], F32)
        nc.vector.tensor_reduce(ksum[0:1, 0:8],
                                kpe[:].rearrange("p (h w) -> p h w", h=8),
                                mybir.AxisListType.X, OP.add)
        nc.vector.reciprocal_approx_fast(ksum[0:1, 8:16], ksum[0:1, 0:8])
        kpn = ph.tile([1, 72], F32)
        for j in range(8):
            nc.vector.tensor_scalar(kpn[0:1, 9 * j:9 * j + 9],
                                    kpe[0:1, 9 * j:9 * j + 9],
                                    ksum[0:1, 8 + j:9 + j], None, OP.mult)
        wvp = ppk.tile([128, 512], F32, tag='k', name="wvp")
        nc.tensor.matmul(wvp[:, 0:72], ones_r[0:1, :], kpn[0:1, :],
                         start=True, stop=True, skip_group_check=True)
        wvec = ph.tile([128, 72], F32)
        nc.vector.tensor_copy(wvec[:], wvp[:, 0:72])

        nc.sync.dma_start(mq[:], mq_d[:])
        nc.sync.dma_start(mb[:], mb_d[:])
        pw = load_kp(pw_d, 4, D, "pw_t")          # [128, 4*1024]
        # ---------------- qkv projections ----------------
        qt = pwp.tile([128, 4 * T], BF16)
        ktp = pwp.tile([128, 4 * KTP], BF16)
        nc.vector.memset(ktp[:], 0.0)
        for t in range(4):
            pq = ppc.tile([128, 1024], F32, tag='c')
            for n in range(2):
                for k in range(8):
                    nc.tensor.matmul(pq[:, 512 * n:512 * n + 512],
                                     wq[:, 512 * k + 128 * t:512 * k + 128 * t + 128],
                                     xt[:, T * k + 512 * n:T * k + 512 * n + 512],
                                     start=(k == 0), stop=(k == 7),
                                     skip_group_check=True)
            nc.scalar.copy(qt[:, T * t:T * t + T], pq[:])
            pk = ppc.tile([128, 1024], F32, tag='c')
            for n in range(2):
                for k in range(8):
                    nc.tensor.matmul(pk[:, 512 * n:512 * n + 512],
                                     wk[:, 512 * k + 128 * t:512 * k + 128 * t + 128],
                                     xt[:, T * k + 512 * n:T * k + 512 * n + 512],
                                     start=(k == 0), stop=(k == 7),
                                     skip_group_check=True)
            nc.scalar.copy(ktp[:, KTP * t + 1:KTP * t + 1 + T], pk[:])
        vpp = pwp.tile([128, NCH * 520], BF16)
        nc.vector.memset(vpp[:], 1.0)
        for c in range(NCH):
            M_c = _chunk_rows(c)
            pv_ = ppc.tile([128, 1024], F32, tag='c')
            for k in range(8):
                nc.tensor.matmul(pv_[0:M_c, 0:512],
                                 xt[:, T * k + CH * c:T * k + CH * c + M_c],
                                 wv[:, 512 * k:512 * k + 512],
                                 start=(k == 0), stop=(k == 7),
                                 skip_group_check=True)
            nc.vector.tensor_copy(
                vpp[0:M_c, 520 * c:520 * c + 520]
                    .rearrange("p (h w) -> p h w", h=8)[:, :, 0:64],
                pv_[0:M_c, 0:512].rearrange("p (h w) -> p h w", h=8))

        if DBG:
            nc.sync.dma_start(dbg["dqt"][:], qt[:])
            nc.sync.dma_start(dbg["dktp"][:], ktp[:])
            nc.sync.dma_start(dbg["deh"][:], eh[:])
            nc.sync.dma_start(dbg["dkpn"][:], kpn[:])
            nc.sync.dma_start(dbg["dvpp"][:], vpp[:])
        # ---------------- attention maps ----------------
        nrm_sem = nc.alloc_semaphore("nrm_sem")
        oT = pwp.tile([128, 4 * T], BF16)
        bds = []
        for j in range(8):
            bd = pbd.tile([128, 3 * 128], BF16, name=f"bd{j}", tag="bd")
            bds.append(bd)
            for dq in range(3):
                nc.vector.tensor_scalar(
                    bd[:, 128 * dq:128 * dq + 128], i3[:, 0:128],
                    wvec[:, 9 * j + 3 * dq:9 * j + 3 * dq + 1], None, OP.mult)
                for d in (1, 2):
                    nc.vector.scalar_tensor_tensor(
                        bd[:, 128 * dq:128 * dq + 128],
                        i3[:, 128 * d:128 * d + 128],
                        wvec[:, 9 * j + 3 * dq + d:9 * j + 3 * dq + d + 1],
                        bd[:, 128 * dq:128 * dq + 128],
                        op0=OP.mult, op1=OP.add)
        def emit_pv(j, ovp, c, M_c, Qs, e_):
            for Q in range(Qs, 4):
                lo, hi = QW * Q, QW * Q + QW
                nc.tensor.matmul(ovp[0:65, lo:hi],
                                 vpp[0:M_c, 520 * c + 65 * j:520 * c + 65 * j + 65],
                                 e_[0:M_c, lo:hi],
                                 start=(c == 0 and Q % 2 == 0),
                                 stop=(c == NCH - 1 and Q % 2 == 1),
                                 skip_group_check=True)

        for j in range(8):
            t, prow = j // 2, 64 * (j % 2)
            bd = bds[j]
            pv_pend = []
            ovp = ppv.tile([65, 1024], F32, tag='v')
            for c in range(NCH):
                M_c, Qs = _chunk_rows(c), QSTART[c]
                lo_all = QW * Qs
                # qk into per-half psum tiles
                halves = {}
                for h2 in (0, 1):
                    h_lo, h_hi = 512 * h2, 512 * h2 + 512
                    lo = max(lo_all, h_lo)
                    if lo >= h_hi:
                        continue
                    qh = ppk.tile([128, 512], F32, tag='k', name="qkp")
                    halves[h2] = (qh, h_lo)
                    nc.tensor.matmul(qh[:, lo - h_lo:512],
                                     ktp[64 * (j % 2):64 * (j % 2) + 64,
                                         KTP * t + CH * c:KTP * t + CH * c + 128],
                                     qt[64 * (j % 2):64 * (j % 2) + 64,
                                        T * t + lo:T * t + h_hi],
                                     start=True, stop=True, skip_group_check=True)
                # Z with causal zero-mask
                z_ = pz.tile([128, 1024], BF16)
                if Qs > 0:
                    nc.vector.memset(z_[:, lo_all - 1:lo_all], 0.0)
                runs = []
                for Q in range(Qs, 4):
                    sl = (QW * Q, QW * Q + QW)
                    qh, h_lo = halves[Q // 2]
                    if (c, Q) in MASK_PAIRS:
                        n = MASK_PAIRS.index((c, Q))
                        nc.vector.tensor_tensor(
                            z_[:, sl[0]:sl[1]],
                            qh[:, sl[0] - h_lo:sl[1] - h_lo],
                            mq[:, QW * n:QW * n + QW], op=OP.mult)
                    else:
                        if runs and runs[-1][1] == sl[0] and (runs[-1][0] // 512) == (sl[0] // 512):
                            runs[-1] = (runs[-1][0], sl[1])
                        else:
                            runs.append(sl)
                for (r0, r1) in runs:
                    qh, h_lo = halves[r0 // 512]
                    nc.scalar.copy(z_[:, r0:r1], qh[:, r0 - h_lo:r1 - h_lo])
                if DBG and j == 0 and c == 0:
                    nc.sync.dma_start(dbg["dz0"][:], z_[:])
                # conv: banded matmuls grouped by lhsT (ldweights reuse)
                cvp = ppc.tile([128, 1024], F32, tag='c')
                first_in_bank = {0: True, 1: True}
                mm_list = []
                for dq in range(3):
                    sh = dq - 1
                    for Q in range(Qs, 4):
                        lo, hi = QW * Q, QW * Q + QW
                        rl, rh, ol = lo + sh, hi + sh, lo
                        if rl < 0:
                            rl, ol = 0, lo + 1
                        if rh > T:
                            rh = T
                        mm_list.append((Q, bd[:, 128 * dq:128 * dq + M_c],
                                        z_[:, rl:rh], (ol, ol + rh - rl)))
                for Q in range(Qs, 4):
                    if (c, Q) in MBIG_PAIRS:
                        n = MBIG_PAIRS.index((c, Q))
                        mm_list.append((Q, i3[:, 0:M_c],
                                        mb[:, QW * n:QW * n + QW],
                                        (QW * Q, QW * Q + QW)))
                last_in_bank = {}
                for idx, (Q, _, _, _) in enumerate(mm_list):
                    last_in_bank[Q // 2] = idx
                for idx, (Q, lhsT, rhs, (ol, oh)) in enumerate(mm_list):
                    bank = Q // 2
                    nc.tensor.matmul(cvp[0:M_c, ol:oh], lhsT, rhs,
                                     start=first_in_bank[bank],
                                     stop=(last_in_bank[bank] == idx),
                                     skip_group_check=True)
                    first_in_bank[bank] = False
                # exp -> E
                e_ = pe_.tile([128, 1024], BF16)
                nc.scalar.activation(e_[0:M_c, lo_all:T], cvp[0:M_c, lo_all:T],
                                     AF.Exp)
                if DBG and j == 0 and c == 0:
                    nc.sync.dma_start(dbg["de0"][:], e_[:])
                pv_pend.append((c, M_c, Qs, e_))
                if len(pv_pend) > 1:
                    emit_pv(j, ovp, *pv_pend.pop(0))
            while pv_pend:
                emit_pv(j, ovp, *pv_pend.pop(0))
            if DBG and j == 0:
                ovsb = ppt.tile([128, 1024], F32, name="ovsbdbg")
                nc.vector.tensor_copy(ovsb[0:65, :], ovp[0:65, 0:T])
                nc.sync.dma_start(dbg["dov"][:], ovsb[0:65, :])
            if j < 7:
                zcache[(j + 1, 0)] = emit_qkz(j + 1, 0)
                zcache[(j + 1, 1)] = emit_qkz(j + 1, 1)
            # normalize -> oT (gpsimd broadcast; explicit sems inside critical)
            ssb = pn.tile([1, 2 * T], F32, name="ssb")
            nc.scalar.copy(ssb[0:1, 0:T], ovp[64:65, 0:T])
            rb = pn.tile([64, T], F32)
            with tc.tile_critical():
                nc.vector.reciprocal_approx_fast(
                    ssb[0:1, T:2 * T], ssb[0:1, 0:T]).then_inc(nrm_sem)
                nc.gpsimd.partition_broadcast(
                    rb[:], ssb[0:1, T:2 * T])._wait_ge(
                    nrm_sem, 2 * j + 1).then_inc(nrm_sem)
                nc.vector.tensor_tensor(
                    oT[prow:prow + 64, T * t:T * t + T],
                    ovp[0:64, 0:T], rb[:], op=OP.mult)._wait_ge(
                    nrm_sem, 2 * j + 2)

        # ---------------- output projection partials ----------------
        for m in range(8):
            pp_ = ppq.tile([128, 1024], F32, tag='q')
            for n in range(2):
                for k in range(4):
                    nc.tensor.matmul(pp_[:, 512 * n:512 * n + 512],
                                     oT[:, T * k + 128 * m:T * k + 128 * m + 128],
                                     pw[:, D * k + 512 * n:D * k + 512 * n + 512],
                                     start=(k == 0), stop=(k == 3),
                                     skip_group_check=True)
            psb = ppt.tile([128, 1024], BF16)
            nc.scalar.copy(psb[:], pp_[:])
            pd = part_d if m < 4 else part2_d
            nc.sync.dma_start(pd[128 * (m % 4):128 * (m % 4) + 128, :], psb[:])

        if DBG:
            nc.sync.dma_start(dbg["doT"][:], oT[:])
            nc.sync.dma_start(dbg["dpart"][0:512, :], part_d[:])
            nc.sync.dma_start(dbg["dpart"][512:1024, :], part2_d[:])
        # ---------------- pair ReduceScatter + output ----------------
        cc_sem = nc.alloc_semaphore("cc_sem")
        od_sem = nc.alloc_semaphore("od_sem")
        with tc.tile_critical():
            nc.gpsimd.collective_compute(
                "ReduceScatter", OP.add,
                replica_groups=[[0, 1], [2, 3], [4, 5], [6, 7]],
                ins=[part_d[:]], outs=[crs_d[0:256, :]],
            ).then_inc(cc_sem)
        with tc.tile_critical():
            nc.gpsimd.collective_compute(
                "ReduceScatter", OP.add,
                replica_groups=[[0, 1], [2, 3], [4, 5], [6, 7]],
                ins=[part2_d[:]], outs=[crs_d[256:512, :]],
            ).then_inc(cc_sem)
            nc.gpsimd.wait_ge(cc_sem, 2)
            nc.gpsimd.dma_start(out_d[:], crs_d[:]).then_inc(od_sem, 16)
            nc.gpsimd.wait_ge(od_sem, 16)

    nc.compile()
    _CACHE["nc"] = nc
    return nc


def _host_inputs(inputs):
    i3, i1f, maskq, mbig = _host_consts()
    x = np.asarray(inputs["x"], np.float32)
    hist = np.asarray(inputs["historical_data"], np.float32)
    g = lambda n: np.asarray(inputs[n], np.float32)
    Wq, Wk, Wv = g("Wq"), g("Wk"), g("Wv")
    pW = g("proj_W"); hW = g("hist_W"); c1 = g("ctx_W1"); c2 = g("ctx_W2")
    k1 = g("kg_W1"); k2 = g("kg_W2")
    in_maps = []
    for core in range(N_CORES):
        b, H2 = core // 2, core % 2
        cols = slice(512 * H2, 512 * H2 + 512)
        in_maps.append({
            "xt": np.ascontiguousarray(x[b].T).astype(bf16),
            "wq": (Wq[:, cols] / 8.0).astype(bf16),
            "wk": Wk[:, cols].astype(bf16),
            "wv": Wv[:, cols].astype(bf16),
            "pw": pW[cols, :].astype(bf16),
            "ht": np.ascontiguousarray(hist[b].T).astype(bf16),
            "hw": hW.astype(bf16),
            "c1": c1.astype(bf16),
            "c2": c2.astype(bf16),
            "k1": k1.astype(bf16),
            "k2": k2[:, 72 * H2:72 * H2 + 72].astype(bf16),
            "i3": i3, "i1f": i1f, "mq": maskq, "mb": mbig,
        })
    return in_maps


def run(inputs, trace=False):
    nc = _build()
    in_maps = _host_inputs(inputs)
    res = run_bass_kernel_spmd(nc, in_maps, list(range(N_CORES)), trace=trace)
    out = np.empty((B, T, D), np.float32)
    for b in range(B):
        ev, od = res.results[2 * b]["out"], res.results[2 * b + 1]["out"]
        out[b, 0:256] = ev[0:256]
        out[b, 256:512] = od[0:256]
        out[b, 512:768] = ev[256:512]
        out[b, 768:1024] = od[256:512]
    return out, res


def kernel(**inputs):
    out, _ = run(inputs, trace=False)
    return out
